# revision 19
# baseline (speedup 1.0000x reference)
"""Trainium2 Bass kernel for multi-head causal self-attention.

Problem: X [4, 2048, 1024] fp32, Wq/Wk/Wv/Wo [1024, 1024], H=16 heads, HD=64.
reference: out = softmax_causal((X@Wq) (X@Wk)^T / 8) (X@Wv) merged @ Wo.

Sharding over 8 NeuronCores: core c handles batch b = c // 2 and head group
hg = c % 2 (8 heads each). Each core computes a partial [2048, 1024] output
(its heads' contribution through Wo's row shard); the host sums the two
partials per batch (the tensor-parallel all-reduce, done during unsharding).

Dataflow (bf16 operands, fp32 PSUM accumulation), software-pipelined so the
ACT-engine exp stream hides behind PE matmuls:
  host marshaling: X passed pre-transposed [d, s]; weights pre-tiled to
       [128, chunk, cols] so every ramp load is a contiguous 2D DMA (the
       XBAR DMA-transpose and strided gathers serialize on the dynamic-DMA
       scratch and cost ~25us of ramp; concurrent XBAR transposes corrupt)
  warm-up: dummy matmuls keep the PE busy while ramp DMAs land so the HAM
       clock gate opens (1.2 -> 2.4 GHz) before real work
  Q^T,K^T [512, 2048] = (W chunk).T @ X^T per q-chunk; V [2048, 8, 72]
       = X^T.T @ Wv with col 64 = ones (softmax denominator trick); all
       projection/out-projection chains are generators advanced ~2 MMs at
       a time between attention iterations (fine-grained PE filler)
  S^T  pair [128k, 2, 512q] psum = K^T.T @ Q^T, both heads emitted
       back-to-back with tile_position (0,0)/(64,0) -> concurrent row
       tiles; fully-masked leading columns skipped
  E^T  = exp(S^T/8 [+ diag mask]) on ACT, one instruction per k-tile
       covering both heads' psum banks -> bf16 SBUF
  O'   [72, 2, 512] psum = [V_h | 1 | pad].T @ E^T accumulated over
       k-tiles, masked columns never written (has_written overwrite)
  norm tensor_copy denominator row to partition 0 (custom DVE ops need
       base-partition-0 APs) -> reciprocal_approx_fast -> gpsimd
       partition_broadcast -> DVE multiply straight off the AV psum
  OUT  [128s, 512c] = O^T.T @ Wo over 4 pc chunks; bf16 out; for the last
       q-chunk the pc-3 contribution goes to OUT2 and is summed on the
       host, keeping the kernel tail to 1-MM chains
"""

import sys

for _p in ("/opt/trn_rl_repo", "/root/.axon_site/_ro/trn_rl_repo"):
    if _p not in sys.path:
        sys.path.insert(0, _p)

import ml_dtypes
import numpy as np

import concourse.bass as bass
import concourse.mybir as mybir
import concourse.tile as tile
from concourse import bacc
from concourse.bass_utils import run_bass_kernel_spmd

F32 = mybir.dt.float32
BF16 = mybir.dt.bfloat16
EXPF = mybir.ActivationFunctionType.Exp

B, S, D, H = 4, 2048, 1024, 16
HD = D // H           # 64
HL = H // 2           # 8 heads per core
DL = HL * HD          # 512 local proj width
NEG = -30000.0        # causal mask additive value (exp underflows to 0)
VW = 72               # AV lhsT width: 64 V cols + ones col + 7 pad
INTERLEAVE = True     # software-pipeline projections into attention
COARSE = True         # filler advances whole chains instead of 2-MM steps


def build_program(s=S, d=D, hl=HL):
    dl = hl * HD
    n_st = s // 128          # s-tiles (128 rows)
    n_dc = d // 128          # d-chunks (projection contraction)
    n_pc = dl // 128         # Q^T/K^T partition chunks (= head pairs)
    n_q = s // 512           # q-chunks
    n_cc = d // 512          # out column chunks

    nc = bacc.Bacc("TRN2", target_bir_lowering=False, debug=False)

    # X arrives host-transposed [d, s]; weights host-tiled to
    # [128, chunk, cols] so every load is a contiguous 2D DMA (no XBAR
    # transpose, no strided gather -- those serialize on the dynamic-DMA
    # scratch and cost ~25us of ramp)
    XT = nc.dram_tensor("XT", [d, s], BF16, kind="ExternalInput")
    WQ = nc.dram_tensor("WQ", [128, n_dc * dl], BF16, kind="ExternalInput")
    WK = nc.dram_tensor("WK", [128, n_dc * dl], BF16, kind="ExternalInput")
    WV = nc.dram_tensor("WV", [128, n_dc * dl], BF16, kind="ExternalInput")
    WO = nc.dram_tensor("WO", [128, n_pc * d], BF16, kind="ExternalInput")
    OUT = nc.dram_tensor("OUT", [s, d], BF16, kind="ExternalOutput")
    # pc-3 contribution of the last q-chunk, summed on the host (keeps the
    # kernel tail to 1-MM chains instead of waiting on an on-chip add)
    OUT2 = nc.dram_tensor("OUT2", [s // 4, d], BF16, kind="ExternalOutput")

    with tile.TileContext(nc) as tc:
        with (
            tc.tile_pool(name="persist", bufs=1) as persist,
            tc.tile_pool(name="fillps", bufs=2, space="PSUM") as fillps,
            tc.tile_pool(name="stps", bufs=2, space="PSUM") as stps,
            tc.tile_pool(name="avps", bufs=1, space="PSUM") as avps,
            tc.tile_pool(name="work", bufs=1) as work,
        ):
            # diagonal causal mask block (keep where q >= k)
            cmask = persist.tile([128, 128], F32)
            nc.gpsimd.memset(cmask[:], 0.0)
            nc.gpsimd.affine_select(
                out=cmask[:], in_=cmask[:],
                compare_op=mybir.AluOpType.is_ge, fill=NEG,
                base=0, pattern=[[1, 128]], channel_multiplier=-1,
            )

            qt = [persist.tile([128, s], BF16, name=f"qt{i}") for i in range(n_pc)]
            kt = [persist.tile([128, s], BF16, name=f"kt{i}") for i in range(n_pc)]
            vt = [persist.tile([128, hl, VW], BF16, name=f"vt{i}") for i in range(n_st)]
            ot = [persist.tile([128, s], BF16, name=f"ot{i}") for i in range(n_pc)]
            xt = persist.tile([128, n_dc, s], BF16, name="xt")
            wq = persist.tile([128, n_dc, dl], BF16, name="wq")
            wk = persist.tile([128, n_dc, dl], BF16, name="wk")
            wv = persist.tile([128, n_dc, dl], BF16, name="wv")
            wo = persist.tile([128, n_pc, d], BF16, name="wo")

            # ---- ramp: all plain contiguous DMAs, split across queues ----
            nc.scalar.dma_start(
                wq[:], WQ.ap().rearrange("p (c m) -> p c m", c=n_dc))
            for dc in range(n_dc):
                nc.sync.dma_start(
                    xt[:, dc, :], XT[dc * 128:(dc + 1) * 128, :])
            nc.scalar.dma_start(
                wk[:], WK.ap().rearrange("p (c m) -> p c m", c=n_dc))
            nc.scalar.dma_start(
                wv[:], WV.ap().rearrange("p (c m) -> p c m", c=n_dc))
            # ones / pad columns of V tiles, set once
            for st in range(n_st):
                nc.gpsimd.memset(vt[st][:, :, HD:VW], 1.0)

            # HAM keep-alive: dummy matmuls keep the PE busy while the ramp
            # DMAs land, so the clock gate opens (1.2 -> 2.4 GHz) before the
            # first projection chain instead of ~40us into the kernel
            warm = work.tile([128, 512], BF16, tag="warm", name="warm")
            nc.vector.memset(warm[:], 0.0)
            wps = fillps.tile([128, 512], F32, tag="fill", name="wps")
            for _ in range(72):
                nc.tensor.matmul(wps[:], warm[:, 0:128], warm[:],
                                 start=True, stop=True)

            # ---------- filler units: generators that yield between small
            # MM groups so they interleave with attention at ~2-MM grain,
            # keeping both PE and ACT pipelines fed ----------
            def qk_unit(jn, pc, w, dst):
                def run():
                    ps = fillps.tile([128, 512], F32, tag="fill", name="psqk")
                    for dc in range(n_dc):
                        nc.tensor.matmul(
                            ps[:], w[:, dc, pc * 128:(pc + 1) * 128],
                            xt[:, dc, jn * 512:(jn + 1) * 512],
                            start=(dc == 0), stop=(dc == n_dc - 1))
                        if not COARSE and dc % 2 == 1 and dc < n_dc - 1:
                            yield
                    nc.vector.tensor_copy(
                        dst[pc][:, jn * 512:(jn + 1) * 512], ps[:])
                return run

            def v_unit(st):
                def run():
                    ps = fillps.tile([128, dl], F32, tag="fill", name="psv")
                    for dc in range(n_dc):
                        nc.tensor.matmul(
                            ps[:], xt[:, dc, st * 128:(st + 1) * 128],
                            wv[:, dc, :],
                            start=(dc == 0), stop=(dc == n_dc - 1))
                        if not COARSE and dc % 2 == 1 and dc < n_dc - 1:
                            yield
                    nc.vector.tensor_copy(
                        vt[st][:, :, 0:HD],
                        ps[:].rearrange("p (h e) -> p h e", h=hl))
                return run

            def proj_units(jn):
                us = []
                for pc in range(n_pc):
                    us.append(qk_unit(jn, pc, wq, qt))
                for pc in range(n_pc):
                    us.append(qk_unit(jn, pc, wk, kt))
                for st in range(4 * jn, 4 * jn + 4):
                    us.append(v_unit(st))
                return us

            def outproj_unit(jo, st, cc):
                def run():
                    ps = fillps.tile([128, 512], F32, tag="fill", name="psop")
                    for pc in range(n_pc):
                        nc.tensor.matmul(
                            ps[:], ot[pc][:, st * 128:(st + 1) * 128],
                            wo[:, pc, cc * 512:(cc + 1) * 512],
                            start=(pc == 0), stop=(pc == n_pc - 1))
                        if not COARSE and pc % 2 == 1 and pc < n_pc - 1:
                            yield
                    osb = work.tile([128, 512], BF16, tag="osb", bufs=3,
                                    name="osb")
                    nc.vector.tensor_copy(osb[:], ps[:])
                    nc.sync.dma_start(
                        OUT[st * 128:(st + 1) * 128,
                            cc * 512:(cc + 1) * 512], osb[:])
                return run

            def outproj_units(jo):
                return [outproj_unit(jo, st, cc)
                        for st in range(4 * jo, 4 * jo + 4)
                        for cc in range(n_cc)]

            # last q-chunk: pc 0..2 partials go straight to OUT; the pc-3
            # contribution lands in OUT2 and the host adds them during the
            # gather. Keeps the kernel tail to 1-MM chains.
            def outproj_partial_unit(st, cc):
                def run():
                    ps = fillps.tile([128, 512], F32, tag="fill", name="psp")
                    for pc in range(n_pc - 1):
                        nc.tensor.matmul(
                            ps[:], ot[pc][:, st * 128:(st + 1) * 128],
                            wo[:, pc, cc * 512:(cc + 1) * 512],
                            start=(pc == 0), stop=(pc == n_pc - 2))
                        if not COARSE and pc == 1:
                            yield
                    osb = work.tile([128, 512], BF16, tag="osb3", bufs=4,
                                    name="osb3")
                    nc.scalar.copy(osb[:], ps[:])
                    nc.sync.dma_start(
                        OUT[st * 128:(st + 1) * 128,
                            cc * 512:(cc + 1) * 512], osb[:])
                return run

            def outproj_final_unit(st, cc):
                def run():
                    ps = fillps.tile([128, 512], F32, tag="fill", name="psf")
                    nc.tensor.matmul(
                        ps[:], ot[n_pc - 1][:, st * 128:(st + 1) * 128],
                        wo[:, n_pc - 1, cc * 512:(cc + 1) * 512],
                        start=True, stop=True)
                    ofin = work.tile([128, 512], BF16, tag="ofin", bufs=3,
                                     name="ofin")
                    nc.scalar.copy(ofin[:], ps[:])
                    nc.sync.dma_start(
                        OUT2[(st - 12) * 128:(st - 11) * 128,
                             cc * 512:(cc + 1) * 512], ofin[:])
                    if False:
                        yield
                return run

            # generator-stream filler: step() advances the current unit
            # generator by one yield-point (~2 MMs)
            class Filler:
                def __init__(self, units):
                    self.units = list(units)
                    self.cur = None

                def step(self, n=1):
                    for _ in range(n):
                        while True:
                            if self.cur is None:
                                if not self.units:
                                    return
                                self.cur = self.units.pop(0)()
                            try:
                                next(self.cur)
                                break
                            except StopIteration:
                                self.cur = None

                def drain(self):
                    while self.units or self.cur is not None:
                        self.step(1)

                def steps_left(self):
                    per = 1 if COARSE else 4
                    return per * len(self.units) + (1 if self.cur else 0)

            # ---- chunk-0 projections run un-interleaved (nothing to hide) ----
            Filler(proj_units(0)).drain()
            nc.sync.dma_start(
                wo[:], WO.ap().rearrange("p (c m) -> p c m", c=n_pc))

            # ---- attention per q-chunk with PE filler interleave ----
            def attention(j, fill_units, front_steps, late_units=()):
                """fill_units: generator units stepped evenly across this
                chunk's iterations. front_steps: steps forced at ~3/iter at
                the start (KV deps for this chunk's diagonal). late_units:
                stepped only during the last pc (depend on earlier pc)."""
                fill = Filler(fill_units)
                late = Filler(late_units)
                n_iter = n_pc * 4 * (j + 1)
                n_last = 4 * (j + 1)
                total_steps = fill.steps_left()
                spread = max(total_steps - front_steps, 0)
                state = {"credit": 0.0, "lcredit": 0.0, "fronted": 0,
                         "it": 0}
                lsteps = late.steps_left()

                def pull(in_last_pc):
                    state["it"] += 1
                    if in_last_pc and lsteps:
                        state["lcredit"] += lsteps / n_last
                        k = int(state["lcredit"])
                        if k:
                            late.step(k)
                            state["lcredit"] -= k
                    if state["fronted"] < front_steps:
                        fill.step(3)
                        state["fronted"] += 3
                        return
                    state["credit"] += spread / max(n_iter, 1)
                    k = int(state["credit"])
                    if k:
                        fill.step(k)
                        state["credit"] -= k

                js = slice(j * 512, (j + 1) * 512)
                n_i = 4 * (j + 1)
                for pc in range(n_pc):
                    av = avps.tile([VW, 2, 512], F32, tag="av",
                                   name=f"av{j}_{pc}")
                    # iterate non-diagonal k-tiles first: they only need
                    # this chunk's Q plus older K/V, so KV filler for this
                    # chunk can still be in flight
                    order = list(range(4 * j)) + list(range(4 * j, n_i))
                    first = order[0]
                    last = order[-1]
                    for i in order:
                        r = i - 4 * j
                        rs = max(r, 0) * 128
                        stp = stps.tile([128, 2, 512], F32, tag="stp",
                                        name=f"stp{j}_{pc}_{i}")
                        for h in (0, 1):
                            hs = slice(64 * h, 64 * h + 64)
                            nc.tensor.matmul(
                                stp[:, h, rs:512],
                                kt[pc][hs, i * 128:(i + 1) * 128],
                                qt[pc][hs, j * 512 + rs:(j + 1) * 512],
                                start=True, stop=True,
                                tile_position=(64 * h, 0))
                        if r >= 0:
                            for h in (0, 1):
                                nc.vector.tensor_add(
                                    stp[:, h, rs:rs + 128],
                                    stp[:, h, rs:rs + 128], cmask[:])
                        et = work.tile([128, 2, 512], BF16, tag="et", bufs=4,
                                       name=f"et{j}_{pc}_{i}")
                        nc.scalar.activation(
                            et[:, :, rs:512], stp[:, :, rs:512], EXPF,
                            scale=0.125)
                        pull(pc == n_pc - 1)
                        for h in (0, 1):
                            nc.tensor.matmul(
                                av[:, h, rs:512], vt[i][:, 2 * pc + h, :],
                                et[:, h, rs:512],
                                start=(i == first), stop=(i == last))
                    # softmax normalization straight off the AV psum.
                    # tensor_copy shifts the denominator row to partition 0
                    # (custom DVE ops require base-partition-0 APs)
                    den = work.tile([1, 2, 512], F32, tag="den", bufs=2,
                                    name=f"den{j}_{pc}")
                    nc.vector.tensor_copy(den[:], av[64:65, :, :])
                    rinv = work.tile([1, 2, 512], F32, tag="rinv", bufs=2,
                                     name=f"rinv{j}_{pc}")
                    nc.vector.reciprocal_approx_fast(rinv[:], den[:])
                    for h in (0, 1):
                        bc = work.tile([64, 512], F32, tag="bc", bufs=2,
                                       name=f"bc{j}_{pc}_{h}")
                        nc.gpsimd.partition_broadcast(bc[:], rinv[0:1, h, :])
                        if h == 0:
                            nc.vector.tensor_mul(
                                ot[pc][0:64, js], av[0:64, h, :], bc[:])
                        else:
                            sc = work.tile([64, 512], BF16, tag="sc", bufs=2,
                                           name=f"sc{j}_{pc}")
                            nc.vector.tensor_mul(
                                sc[:], av[0:64, h, :], bc[:])
                            nc.gpsimd.dma_start(ot[pc][64:128, js], sc[:])
                # drain remaining filler
                fill.drain()
                late.drain()

            if INTERLEAVE:
                attention(0, proj_units(1), front_steps=0)
                attention(1, proj_units(2), front_steps=0)
                attention(2, (proj_units(3)[:4]           # Q(3)
                              + outproj_units(0) + outproj_units(1)),
                          front_steps=0)
                fill3 = (proj_units(3)[4:]                # K(3), V(3)
                         + outproj_units(2))
                late3 = [outproj_partial_unit(st, cc)
                         for st in range(12, 16) for cc in range(n_cc)]
                attention(3, fill3, front_steps=(8 if COARSE else 32), late_units=late3)
                for st in range(12, 16):
                    for cc in range(n_cc):
                        for _ in outproj_final_unit(st, cc)():
                            pass
            else:
                attention(0, [], front_steps=0)
                Filler(proj_units(1)).drain()
                attention(1, [], front_steps=0)
                Filler(proj_units(2)).drain()
                attention(2, [], front_steps=0)
                Filler(proj_units(3)).drain()
                attention(3, [], front_steps=0)
                for jo in range(4):
                    Filler(outproj_units(jo)).drain()

    nc.compile()
    return nc


_NC_CACHE = {}


def _get_program():
    key = (S, D, HL)
    if key not in _NC_CACHE:
        _NC_CACHE[key] = build_program()
    return _NC_CACHE[key]


def _bf16(a):
    return np.ascontiguousarray(a.astype(ml_dtypes.bfloat16))


def _tile_w(w, chunks):
    # [rows, cols] -> [128, chunks*cols] with rows split as (chunks, 128)
    rows, cols = w.shape
    assert rows == chunks * 128
    return _bf16(w.reshape(chunks, 128, cols).transpose(1, 0, 2)
                 .reshape(128, chunks * cols))


def make_in_maps(X, Wq, Wk, Wv, Wo):
    in_maps = []
    for c in range(8):
        b, hg = c // 2, c % 2
        cs = slice(hg * DL, hg * DL + DL)
        in_maps.append({
            "XT": _bf16(X[b].T),
            "WQ": _tile_w(Wq[:, cs], 8),
            "WK": _tile_w(Wk[:, cs], 8),
            "WV": _tile_w(Wv[:, cs], 8),
            "WO": _tile_w(Wo[cs, :], 4),
        })
    return in_maps


def gather_out(results):
    out = np.empty((B, S, D), dtype=np.float32)
    for b in range(B):
        out[b] = (results[2 * b]["OUT"].astype(np.float32)
                  + results[2 * b + 1]["OUT"].astype(np.float32))
        out[b][3 * S // 4:] += (
            results[2 * b]["OUT2"].astype(np.float32)
            + results[2 * b + 1]["OUT2"].astype(np.float32))
    return out


def kernel(X, Wq, Wk, Wv, Wo):
    X = np.asarray(X, dtype=np.float32)
    Wq = np.asarray(Wq, dtype=np.float32)
    Wk = np.asarray(Wk, dtype=np.float32)
    Wv = np.asarray(Wv, dtype=np.float32)
    Wo = np.asarray(Wo, dtype=np.float32)

    nc = _get_program()
    in_maps = make_in_maps(X, Wq, Wk, Wv, Wo)
    res = run_bass_kernel_spmd(nc, in_maps, list(range(8)), trace=False)
    return gather_out(res.results)


if __name__ == "__main__":
    rng = np.random.default_rng(0)
    scale = 1.0 / np.sqrt(D)
    inputs = {
        "X": rng.standard_normal((B, S, D), dtype=np.float32),
        "Wq": rng.standard_normal((D, D), dtype=np.float32) * scale,
        "Wk": rng.standard_normal((D, D), dtype=np.float32) * scale,
        "Wv": rng.standard_normal((D, D), dtype=np.float32) * scale,
        "Wo": rng.standard_normal((D, D), dtype=np.float32) * scale,
    }
    out = kernel(**inputs)
    print("kernel output shape:", out.shape)


# revision 20
# speedup vs baseline: 1.0513x; 1.0513x over previous
"""Trainium2 Bass kernel for multi-head causal self-attention.

Problem: X [4, 2048, 1024] fp32, Wq/Wk/Wv/Wo [1024, 1024], H=16 heads, HD=64.
reference: out = softmax_causal((X@Wq) (X@Wk)^T / 8) (X@Wv) merged @ Wo.

Sharding over 8 NeuronCores: core c handles batch b = c // 2 and head group
hg = c % 2 (8 heads each). Each core computes a partial [2048, 1024] output
(its heads' contribution through Wo's row shard); the host sums the two
partials per batch (the tensor-parallel all-reduce, done during unsharding).

Dataflow (bf16 operands, fp32 PSUM accumulation), software-pipelined so the
ACT-engine exp stream hides behind PE matmuls:
  host marshaling: X passed pre-transposed [d, s]; weights pre-tiled to
       [128, chunk, cols] so every ramp load is a contiguous 2D DMA (the
       XBAR DMA-transpose and strided gathers serialize on the dynamic-DMA
       scratch and cost ~25us of ramp; concurrent XBAR transposes corrupt)
  warm-up: dummy matmuls keep the PE busy while ramp DMAs land so the HAM
       clock gate opens (1.2 -> 2.4 GHz) before real work
  Q^T,K^T [512, 2048] = (W chunk).T @ X^T per q-chunk; V [2048, 8, 72]
       = X^T.T @ Wv with col 64 = ones (softmax denominator trick); all
       projection/out-projection chains are generators advanced ~2 MMs at
       a time between attention iterations (fine-grained PE filler)
  S^T  pair [128k, 2, 512q] psum = K^T.T @ Q^T, both heads emitted
       back-to-back with tile_position (0,0)/(64,0) -> concurrent row
       tiles; fully-masked leading columns skipped
  E^T  = exp(S^T/8 [+ diag mask]) on ACT, one instruction per k-tile
       covering both heads' psum banks -> bf16 SBUF
  O'   [72, 2, 512] psum = [V_h | 1 | pad].T @ E^T accumulated over
       k-tiles, masked columns never written (has_written overwrite)
  norm tensor_copy denominator row to partition 0 (custom DVE ops need
       base-partition-0 APs) -> reciprocal_approx_fast -> gpsimd
       partition_broadcast -> DVE multiply straight off the AV psum
  OUT  [128s, 512c] = O^T.T @ Wo over 4 pc chunks; bf16 out; for the last
       q-chunk the pc-3 contribution goes to OUT2 and is summed on the
       host, keeping the kernel tail to 1-MM chains
"""

import sys

for _p in ("/opt/trn_rl_repo", "/root/.axon_site/_ro/trn_rl_repo"):
    if _p not in sys.path:
        sys.path.insert(0, _p)

import ml_dtypes
import numpy as np

import concourse.bass as bass
import concourse.mybir as mybir
import concourse.tile as tile
from concourse import bacc
from concourse.bass_utils import run_bass_kernel_spmd

F32 = mybir.dt.float32
BF16 = mybir.dt.bfloat16
EXPF = mybir.ActivationFunctionType.Exp

B, S, D, H = 4, 2048, 1024, 16
HD = D // H           # 64
HL = H // 2           # 8 heads per core
DL = HL * HD          # 512 local proj width
NEG = -30000.0        # causal mask additive value (exp underflows to 0)
VW = 72               # AV lhsT width: 64 V cols + ones col + 7 pad
INTERLEAVE = True     # software-pipeline projections into attention
COARSE = False        # fine-grained (~2-MM) filler steps measured fastest


def build_program(s=S, d=D, hl=HL):
    dl = hl * HD
    n_st = s // 128          # s-tiles (128 rows)
    n_dc = d // 128          # d-chunks (projection contraction)
    n_pc = dl // 128         # Q^T/K^T partition chunks (= head pairs)
    n_q = s // 512           # q-chunks
    n_cc = d // 512          # out column chunks

    nc = bacc.Bacc("TRN2", target_bir_lowering=False, debug=False)

    # X arrives host-transposed [d, s]; weights host-tiled to
    # [128, chunk, cols] so every load is a contiguous 2D DMA (no XBAR
    # transpose, no strided gather -- those serialize on the dynamic-DMA
    # scratch and cost ~25us of ramp)
    XT = nc.dram_tensor("XT", [d, s], BF16, kind="ExternalInput")
    WQ = nc.dram_tensor("WQ", [128, n_dc * dl], BF16, kind="ExternalInput")
    WK = nc.dram_tensor("WK", [128, n_dc * dl], BF16, kind="ExternalInput")
    WV = nc.dram_tensor("WV", [128, n_dc * dl], BF16, kind="ExternalInput")
    WO = nc.dram_tensor("WO", [128, n_pc * d], BF16, kind="ExternalInput")
    OUT = nc.dram_tensor("OUT", [s, d], BF16, kind="ExternalOutput")
    # pc-3 contribution of the last q-chunk, summed on the host (keeps the
    # kernel tail to 1-MM chains instead of waiting on an on-chip add)
    OUT2 = nc.dram_tensor("OUT2", [s // 4, d], BF16, kind="ExternalOutput")

    with tile.TileContext(nc) as tc:
        with (
            tc.tile_pool(name="persist", bufs=1) as persist,
            tc.tile_pool(name="fillps", bufs=2, space="PSUM") as fillps,
            tc.tile_pool(name="stps", bufs=2, space="PSUM") as stps,
            tc.tile_pool(name="avps", bufs=1, space="PSUM") as avps,
            tc.tile_pool(name="work", bufs=1) as work,
        ):
            # diagonal causal mask block (keep where q >= k)
            cmask = persist.tile([128, 128], F32)
            nc.gpsimd.memset(cmask[:], 0.0)
            nc.gpsimd.affine_select(
                out=cmask[:], in_=cmask[:],
                compare_op=mybir.AluOpType.is_ge, fill=NEG,
                base=0, pattern=[[1, 128]], channel_multiplier=-1,
            )

            qt = [persist.tile([128, s], BF16, name=f"qt{i}") for i in range(n_pc)]
            kt = [persist.tile([128, s], BF16, name=f"kt{i}") for i in range(n_pc)]
            vt = [persist.tile([128, hl, VW], BF16, name=f"vt{i}") for i in range(n_st)]
            ot = [persist.tile([128, s], BF16, name=f"ot{i}") for i in range(n_pc)]
            xt = persist.tile([128, n_dc, s], BF16, name="xt")
            wq = persist.tile([128, n_dc, dl], BF16, name="wq")
            wk = persist.tile([128, n_dc, dl], BF16, name="wk")
            wv = persist.tile([128, n_dc, dl], BF16, name="wv")
            wo = persist.tile([128, n_pc, d], BF16, name="wo")

            # ---- ramp: all plain contiguous DMAs, split across queues ----
            nc.scalar.dma_start(
                wq[:], WQ.ap().rearrange("p (c m) -> p c m", c=n_dc))
            for dc in range(n_dc):
                nc.sync.dma_start(
                    xt[:, dc, :], XT[dc * 128:(dc + 1) * 128, :])
            nc.scalar.dma_start(
                wk[:], WK.ap().rearrange("p (c m) -> p c m", c=n_dc))
            nc.scalar.dma_start(
                wv[:], WV.ap().rearrange("p (c m) -> p c m", c=n_dc))
            # ones / pad columns of V tiles, set once
            for st in range(n_st):
                nc.gpsimd.memset(vt[st][:, :, HD:VW], 1.0)

            # HAM keep-alive: dummy matmuls keep the PE busy while the ramp
            # DMAs land, so the clock gate opens (1.2 -> 2.4 GHz) before the
            # first projection chain instead of ~40us into the kernel
            warm = work.tile([128, 512], BF16, tag="warm", name="warm")
            nc.vector.memset(warm[:], 0.0)
            wps = fillps.tile([128, 512], F32, tag="fill", name="wps")
            for _ in range(72):
                nc.tensor.matmul(wps[:], warm[:, 0:128], warm[:],
                                 start=True, stop=True)

            # ---------- filler units: generators that yield between small
            # MM groups so they interleave with attention at ~2-MM grain,
            # keeping both PE and ACT pipelines fed ----------
            def qk_unit(jn, pc, w, dst):
                def run():
                    ps = fillps.tile([128, 512], F32, tag="fill", name="psqk")
                    for dc in range(n_dc):
                        nc.tensor.matmul(
                            ps[:], w[:, dc, pc * 128:(pc + 1) * 128],
                            xt[:, dc, jn * 512:(jn + 1) * 512],
                            start=(dc == 0), stop=(dc == n_dc - 1))
                        if not COARSE and dc % 2 == 1 and dc < n_dc - 1:
                            yield
                    nc.vector.tensor_copy(
                        dst[pc][:, jn * 512:(jn + 1) * 512], ps[:])
                return run

            def v_unit(st):
                def run():
                    ps = fillps.tile([128, dl], F32, tag="fill", name="psv")
                    for dc in range(n_dc):
                        nc.tensor.matmul(
                            ps[:], xt[:, dc, st * 128:(st + 1) * 128],
                            wv[:, dc, :],
                            start=(dc == 0), stop=(dc == n_dc - 1))
                        if not COARSE and dc % 2 == 1 and dc < n_dc - 1:
                            yield
                    nc.vector.tensor_copy(
                        vt[st][:, :, 0:HD],
                        ps[:].rearrange("p (h e) -> p h e", h=hl))
                return run

            def proj_units(jn):
                us = []
                for pc in range(n_pc):
                    us.append(qk_unit(jn, pc, wq, qt))
                for pc in range(n_pc):
                    us.append(qk_unit(jn, pc, wk, kt))
                for st in range(4 * jn, 4 * jn + 4):
                    us.append(v_unit(st))
                return us

            def outproj_unit(jo, st, cc):
                def run():
                    ps = fillps.tile([128, 512], F32, tag="fill", name="psop")
                    for pc in range(n_pc):
                        nc.tensor.matmul(
                            ps[:], ot[pc][:, st * 128:(st + 1) * 128],
                            wo[:, pc, cc * 512:(cc + 1) * 512],
                            start=(pc == 0), stop=(pc == n_pc - 1))
                        if not COARSE and pc % 2 == 1 and pc < n_pc - 1:
                            yield
                    osb = work.tile([128, 512], BF16, tag="osb", bufs=3,
                                    name="osb")
                    nc.vector.tensor_copy(osb[:], ps[:])
                    nc.sync.dma_start(
                        OUT[st * 128:(st + 1) * 128,
                            cc * 512:(cc + 1) * 512], osb[:])
                return run

            def outproj_units(jo):
                return [outproj_unit(jo, st, cc)
                        for st in range(4 * jo, 4 * jo + 4)
                        for cc in range(n_cc)]

            # last q-chunk: pc 0..2 partials go straight to OUT; the pc-3
            # contribution lands in OUT2 and the host adds them during the
            # gather. Keeps the kernel tail to 1-MM chains.
            def outproj_partial_unit(st, cc):
                def run():
                    ps = fillps.tile([128, 512], F32, tag="fill", name="psp")
                    for pc in range(n_pc - 1):
                        nc.tensor.matmul(
                            ps[:], ot[pc][:, st * 128:(st + 1) * 128],
                            wo[:, pc, cc * 512:(cc + 1) * 512],
                            start=(pc == 0), stop=(pc == n_pc - 2))
                        if not COARSE and pc == 1:
                            yield
                    osb = work.tile([128, 512], BF16, tag="osb3", bufs=4,
                                    name="osb3")
                    nc.scalar.copy(osb[:], ps[:])
                    nc.sync.dma_start(
                        OUT[st * 128:(st + 1) * 128,
                            cc * 512:(cc + 1) * 512], osb[:])
                return run

            def outproj_final_unit(st, cc):
                def run():
                    ps = fillps.tile([128, 512], F32, tag="fill", name="psf")
                    nc.tensor.matmul(
                        ps[:], ot[n_pc - 1][:, st * 128:(st + 1) * 128],
                        wo[:, n_pc - 1, cc * 512:(cc + 1) * 512],
                        start=True, stop=True)
                    ofin = work.tile([128, 512], BF16, tag="ofin", bufs=3,
                                     name="ofin")
                    nc.scalar.copy(ofin[:], ps[:])
                    nc.sync.dma_start(
                        OUT2[(st - 12) * 128:(st - 11) * 128,
                             cc * 512:(cc + 1) * 512], ofin[:])
                    if False:
                        yield
                return run

            # generator-stream filler: step() advances the current unit
            # generator by one yield-point (~2 MMs)
            class Filler:
                def __init__(self, units):
                    self.units = list(units)
                    self.cur = None

                def step(self, n=1):
                    for _ in range(n):
                        while True:
                            if self.cur is None:
                                if not self.units:
                                    return
                                self.cur = self.units.pop(0)()
                            try:
                                next(self.cur)
                                break
                            except StopIteration:
                                self.cur = None

                def drain(self):
                    while self.units or self.cur is not None:
                        self.step(1)

                def steps_left(self):
                    per = 1 if COARSE else 4
                    return per * len(self.units) + (1 if self.cur else 0)

            # ---- chunk-0 projections run un-interleaved (nothing to hide) ----
            Filler(proj_units(0)).drain()
            nc.sync.dma_start(
                wo[:], WO.ap().rearrange("p (c m) -> p c m", c=n_pc))

            # ---- attention per q-chunk with PE filler interleave ----
            def attention(j, fill_units, front_steps, late_units=()):
                """fill_units: generator units stepped evenly across this
                chunk's iterations. front_steps: steps forced at ~3/iter at
                the start (KV deps for this chunk's diagonal). late_units:
                stepped only during the last pc (depend on earlier pc)."""
                fill = Filler(fill_units)
                late = Filler(late_units)
                n_iter = n_pc * 4 * (j + 1)
                n_last = 4 * (j + 1)
                total_steps = fill.steps_left()
                spread = max(total_steps - front_steps, 0)
                state = {"credit": 0.0, "lcredit": 0.0, "fronted": 0,
                         "it": 0}
                lsteps = late.steps_left()

                def pull(in_last_pc):
                    state["it"] += 1
                    if in_last_pc and lsteps:
                        state["lcredit"] += lsteps / n_last
                        k = int(state["lcredit"])
                        if k:
                            late.step(k)
                            state["lcredit"] -= k
                    if state["fronted"] < front_steps:
                        fill.step(3)
                        state["fronted"] += 3
                        return
                    state["credit"] += spread / max(n_iter, 1)
                    k = int(state["credit"])
                    if k:
                        fill.step(k)
                        state["credit"] -= k

                js = slice(j * 512, (j + 1) * 512)
                n_i = 4 * (j + 1)
                for pc in range(n_pc):
                    av = avps.tile([VW, 2, 512], F32, tag="av",
                                   name=f"av{j}_{pc}")
                    # iterate non-diagonal k-tiles first: they only need
                    # this chunk's Q plus older K/V, so KV filler for this
                    # chunk can still be in flight
                    order = list(range(4 * j)) + list(range(4 * j, n_i))
                    first = order[0]
                    last = order[-1]
                    for i in order:
                        r = i - 4 * j
                        rs = max(r, 0) * 128
                        stp = stps.tile([128, 2, 512], F32, tag="stp",
                                        name=f"stp{j}_{pc}_{i}")
                        for h in (0, 1):
                            hs = slice(64 * h, 64 * h + 64)
                            nc.tensor.matmul(
                                stp[:, h, rs:512],
                                kt[pc][hs, i * 128:(i + 1) * 128],
                                qt[pc][hs, j * 512 + rs:(j + 1) * 512],
                                start=True, stop=True,
                                tile_position=(64 * h, 0))
                        if r >= 0:
                            for h in (0, 1):
                                nc.vector.tensor_add(
                                    stp[:, h, rs:rs + 128],
                                    stp[:, h, rs:rs + 128], cmask[:])
                        et = work.tile([128, 2, 512], BF16, tag="et", bufs=4,
                                       name=f"et{j}_{pc}_{i}")
                        nc.scalar.activation(
                            et[:, :, rs:512], stp[:, :, rs:512], EXPF,
                            scale=0.125)
                        pull(pc == n_pc - 1)
                        for h in (0, 1):
                            nc.tensor.matmul(
                                av[:, h, rs:512], vt[i][:, 2 * pc + h, :],
                                et[:, h, rs:512],
                                start=(i == first), stop=(i == last))
                    # softmax normalization straight off the AV psum.
                    # tensor_copy shifts the denominator row to partition 0
                    # (custom DVE ops require base-partition-0 APs)
                    den = work.tile([1, 2, 512], F32, tag="den", bufs=2,
                                    name=f"den{j}_{pc}")
                    nc.vector.tensor_copy(den[:], av[64:65, :, :])
                    rinv = work.tile([1, 2, 512], F32, tag="rinv", bufs=2,
                                     name=f"rinv{j}_{pc}")
                    nc.vector.reciprocal_approx_fast(rinv[:], den[:])
                    for h in (0, 1):
                        bc = work.tile([64, 512], F32, tag="bc", bufs=2,
                                       name=f"bc{j}_{pc}_{h}")
                        nc.gpsimd.partition_broadcast(bc[:], rinv[0:1, h, :])
                        if h == 0:
                            nc.vector.tensor_mul(
                                ot[pc][0:64, js], av[0:64, h, :], bc[:])
                        else:
                            sc = work.tile([64, 512], BF16, tag="sc", bufs=2,
                                           name=f"sc{j}_{pc}")
                            nc.vector.tensor_mul(
                                sc[:], av[0:64, h, :], bc[:])
                            nc.gpsimd.dma_start(ot[pc][64:128, js], sc[:])
                # drain remaining filler
                fill.drain()
                late.drain()

            if INTERLEAVE:
                attention(0, proj_units(1), front_steps=0)
                attention(1, proj_units(2), front_steps=0)
                attention(2, (proj_units(3)[:4]           # Q(3)
                              + outproj_units(0) + outproj_units(1)),
                          front_steps=0)
                fill3 = (proj_units(3)[4:]                # K(3), V(3)
                         + outproj_units(2))
                late3 = [outproj_partial_unit(st, cc)
                         for st in range(12, 16) for cc in range(n_cc)]
                attention(3, fill3, front_steps=(8 if COARSE else 32), late_units=late3)
                for st in range(12, 16):
                    for cc in range(n_cc):
                        for _ in outproj_final_unit(st, cc)():
                            pass
            else:
                attention(0, [], front_steps=0)
                Filler(proj_units(1)).drain()
                attention(1, [], front_steps=0)
                Filler(proj_units(2)).drain()
                attention(2, [], front_steps=0)
                Filler(proj_units(3)).drain()
                attention(3, [], front_steps=0)
                for jo in range(4):
                    Filler(outproj_units(jo)).drain()

    nc.compile()
    return nc


_NC_CACHE = {}


def _get_program():
    key = (S, D, HL)
    if key not in _NC_CACHE:
        _NC_CACHE[key] = build_program()
    return _NC_CACHE[key]


def _bf16(a):
    return np.ascontiguousarray(a.astype(ml_dtypes.bfloat16))


def _tile_w(w, chunks):
    # [rows, cols] -> [128, chunks*cols] with rows split as (chunks, 128)
    rows, cols = w.shape
    assert rows == chunks * 128
    return _bf16(w.reshape(chunks, 128, cols).transpose(1, 0, 2)
                 .reshape(128, chunks * cols))


def make_in_maps(X, Wq, Wk, Wv, Wo):
    in_maps = []
    for c in range(8):
        b, hg = c // 2, c % 2
        cs = slice(hg * DL, hg * DL + DL)
        in_maps.append({
            "XT": _bf16(X[b].T),
            "WQ": _tile_w(Wq[:, cs], 8),
            "WK": _tile_w(Wk[:, cs], 8),
            "WV": _tile_w(Wv[:, cs], 8),
            "WO": _tile_w(Wo[cs, :], 4),
        })
    return in_maps


def gather_out(results):
    out = np.empty((B, S, D), dtype=np.float32)
    for b in range(B):
        out[b] = (results[2 * b]["OUT"].astype(np.float32)
                  + results[2 * b + 1]["OUT"].astype(np.float32))
        out[b][3 * S // 4:] += (
            results[2 * b]["OUT2"].astype(np.float32)
            + results[2 * b + 1]["OUT2"].astype(np.float32))
    return out


def kernel(X, Wq, Wk, Wv, Wo):
    X = np.asarray(X, dtype=np.float32)
    Wq = np.asarray(Wq, dtype=np.float32)
    Wk = np.asarray(Wk, dtype=np.float32)
    Wv = np.asarray(Wv, dtype=np.float32)
    Wo = np.asarray(Wo, dtype=np.float32)

    nc = _get_program()
    in_maps = make_in_maps(X, Wq, Wk, Wv, Wo)
    res = run_bass_kernel_spmd(nc, in_maps, list(range(8)), trace=False)
    return gather_out(res.results)


if __name__ == "__main__":
    rng = np.random.default_rng(0)
    scale = 1.0 / np.sqrt(D)
    inputs = {
        "X": rng.standard_normal((B, S, D), dtype=np.float32),
        "Wq": rng.standard_normal((D, D), dtype=np.float32) * scale,
        "Wk": rng.standard_normal((D, D), dtype=np.float32) * scale,
        "Wv": rng.standard_normal((D, D), dtype=np.float32) * scale,
        "Wo": rng.standard_normal((D, D), dtype=np.float32) * scale,
    }
    out = kernel(**inputs)
    print("kernel output shape:", out.shape)
